# revision 30
# baseline (speedup 1.0000x reference)
"""Causal single-head attention (d_model-wide) for Trainium2, 8-core SPMD.

Problem: B=4, S=2048, D=1024; Q/K/V = x@W{q,k,v}.T; scores=Q@K.T (causal,
scale 1/sqrt(D)); out = softmax(scores)@V @ Wo.T -> [B, S, 64].

Sharding (profile-uniform causal blocks): each batch b has 16 query blocks of
128 rows; block j needs j+1 key chunks (causal). Core 2b gets the odd-count
blocks {0,2,...,14}, core 2b+1 the even-count blocks {1,3,...,15}; counts
round up to the even profile P in {2,4,...,16}, so EVERY core runs the same
chunk schedule (one SPMD program) while skipping all score blocks above the
causal diagonal. Blocks pair as (P, P+2) into 256-wide matmul tiles: chunks
< P serve both halves (N=256), the last two chunks serve only the second
half (N=128). The only per-core differences are data: which x rows feed Q,
and two [128,256] fp16 multiplicative masks m0/m1 (triangle/ones/zeros)
applied post-exp at each block's last two chunks.

On-device layout (no transposes anywhere; host pre-transposes x and W):
  KT[e, k] = sum_d WkT[d,e] xT[d,k]      (lhsT=WkT tile, rhs=xT tile)
  QT[e, q] likewise; V[k, e] = sum_d xT[d,k] WvT[d,e] (lhsT=xT, rhs=WvT)
  ST[k, q] = sum_e KT[e,k] QT[e,q]       (scores, transposed; PSUM f32)
  PT[k, q] = exp(ST/32) * mask           (ACT exp, fp16; no max-subtraction:
                                          |ST|/32 <~ 8 so exp is fp32-safe)
  ctxT[e, q] += sum_k V[k,e] PT[k,q]     (PSUM f32 over k chunks; 2 e-slices
                                          per 2KB bank: one start/stop per
                                          bank since start zeroes the bank)
  sums[q]   += sum_k PT[k,q]             (ones-matmul into PSUM)
  out[q, v] = (sum_e ctxT[e,q] WoT[e,v]) * (1/sums[q])

Everything is fp16 storage (inputs converted on host) with fp32 PSUM
accumulation; measured rel err vs the fp32 reference is ~5e-4.
"""

import numpy as np

B, S, D, DV = 4, 2048, 1024, 64
NQ = S // 2          # queries per core
ET = D // 128        # 8 e-tiles
DT = D // 128        # 8 d-tiles
KT_N = S // 128      # 16 key tiles of 128
PCHUNK = 512         # projection free-dim chunk
QB = 256             # query block (free dim of attention matmuls)
N_QB = NQ // QB      # 4
SCALE = 1.0 / 32.0   # 1/sqrt(D)
NEG = -1.0e30

_cache = {}


def _patch_drain_split(tile_mod, mybir, ScopedClock):
    """This walrus build accepts only ONE sync-wait on the kernel-tail SP
    Drain; split the waits across multiple drain instructions."""
    if getattr(tile_mod.TileContext, "_drain_split_patched", False):
        return

    def _patched(self, tick_clock, wait_clock):
        nc = self.nc
        drain_inst = nc.sync.drain()
        wait_clock.add_sem_waits(
            drain_inst.ins, ScopedClock({None: tick_clock.global_clock})
        )
        ins = drain_inst.ins
        waits = list(ins.sync_info.on_wait or [])
        if len(waits) > 1:
            ins.sync_info = mybir.SyncInfo(
                on_wait=waits[:1], on_update=list(ins.sync_info.on_update or [])
            )
            for w in waits[1:]:
                extra = nc.sync.drain()
                extra.ins.sync_info = mybir.SyncInfo(on_wait=[w], on_update=[])
        nc.all_engine_barrier()
        assert self.sems is not None
        popped = nc._tile_sem_poison_stack.pop()
        assert popped is self._sem_poison
        nc.clear_and_free_semaphores(list(self.sems.allocated().values()))
        nc.all_engine_barrier()

    tile_mod.TileContext._drain_and_barrier = _patched
    tile_mod.TileContext._drain_split_patched = True


def _split_multiwait_bir(bir_json):
    """This walrus build accepts only one sync-wait per instruction. Rewrite
    the BIR so any instruction with N>1 waits is preceded by N-1 single-wait
    Drain instructions on the same engine (engine streams execute in block
    order, so the waits are enforced before the instruction issues)."""
    import json

    m = json.loads(bir_json)
    changed = False
    for fn in m.get("functions", []):
        for blk in fn.get("blocks", []):
            insts = blk.get("instructions", [])
            out = []
            for inst in insts:
                si = inst.get("sync_info")
                waits = (si or {}).get("on_wait") or []
                if len(waits) > 1:
                    changed = True
                    for i, w in enumerate(waits[:-1]):
                        out.append(
                            {
                                "name": f"{inst['name']}-sw{i}",
                                "opcode": "Drain",
                                "engine": inst["engine"],
                                "ins": [],
                                "outs": [],
                                "is_reset_sema": False,
                                "sync_info": {"on_wait": [w], "on_update": []},
                                "debug": inst.get("debug"),
                            }
                        )
                    si["on_wait"] = [waits[-1]]
                out.append(inst)
            blk["instructions"] = out
    if not changed:
        return bir_json
    return json.dumps(m).encode()


def _patch_compile_hook():
    """Route every BIR compile through _split_multiwait_bir."""
    import concourse.bass_utils as bu

    if getattr(bu, "_multiwait_patched", False):
        return
    orig = bu.compile_bir_kernel

    def wrapped(bir_json, tmpdir, neff_name="file.neff"):
        return orig(_split_multiwait_bir(bir_json), tmpdir, neff_name)

    bu.compile_bir_kernel = wrapped
    bu._multiwait_patched = True
    try:
        import concourse.bass2jax as b2j

        b2j.compile_bir_kernel = wrapped
    except Exception:
        pass


def _install_ntff_hook():
    """Provide antenv.axon_hooks (absent in this image) so that
    run_bass_kernel_spmd(trace=True) can profile through axon."""
    import sys
    import types

    if "antenv.axon_hooks" in sys.modules:
        return
    try:
        import trn_agent_boot.trn_boot as tb

        mod = types.ModuleType("antenv.axon_hooks")
        _hook = [None]
        mod.set_axon_ntff_profile_hook = lambda h: _hook.__setitem__(0, h)
        mod.get_axon_ntff_profile_hook = lambda: _hook[0]
        sys.modules["antenv.axon_hooks"] = mod
        mod.set_axon_ntff_profile_hook(
            tb._ntff_profile_via_ctypes("/opt/axon/libaxon_pjrt.so")
        )
    except Exception:
        pass


def _build():
    if "nc" in _cache:
        return _cache["nc"]

    import concourse.bass as bass
    import concourse.mybir as mybir
    import concourse.tile as tile
    from concourse.vector_clock import ScopedClock

    _patch_drain_split(tile, mybir, ScopedClock)
    _patch_compile_hook()
    _install_ntff_hook()

    f32 = mybir.dt.float32
    f32r = mybir.dt.float32r
    f16 = mybir.dt.float16

    nc = bass.Bass()
    xkvT = nc.dram_tensor("xkvT", [D, S], f16, kind="ExternalInput")
    xqT = nc.dram_tensor("xqT", [D, NQ], f16, kind="ExternalInput")
    wkT = nc.dram_tensor("wkT", [D, D], f16, kind="ExternalInput")
    wvT = nc.dram_tensor("wvT", [D, D], f16, kind="ExternalInput")
    wqT = nc.dram_tensor("wqT", [D, D], f16, kind="ExternalInput")
    woT = nc.dram_tensor("woT", [D, DV], f16, kind="ExternalInput")
    m0f = nc.dram_tensor("m0f", [128, QB], f16, kind="ExternalInput")
    m1f = nc.dram_tensor("m1f", [128, QB], f16, kind="ExternalInput")
    out = nc.dram_tensor("out", [NQ, DV], f32, kind="ExternalOutput")

    with tile.TileContext(nc) as tc:
        with (
            tc.tile_pool(name="kt", bufs=1) as ktp,
            tc.tile_pool(name="v", bufs=1) as vp,
            tc.tile_pool(name="qt", bufs=1) as qtp,
            tc.tile_pool(name="small", bufs=1) as small,
            tc.tile_pool(name="attn_sb", bufs=4) as attn_sb,
            tc.tile_pool(name="ctxsb", bufs=2) as ctxsbp,
            tc.tile_pool(name="osb", bufs=2) as osbp,
        ):
            # persistent fp16 operand stores
            kt_sb = [ktp.tile([128, S], f16, tag=f"kt{e}", name=f"kt{e}") for e in range(ET)]
            v_sb = [vp.tile([128, D], f16, tag=f"v{k}", name=f"v{k}") for k in range(KT_N)]
            qt_sb = [qtp.tile([128, NQ], f16, tag=f"qt{e}", name=f"qt{e}") for e in range(ET)]

            ones_sb = small.tile([128, 1], f16, name="ones")
            nc.vector.memset(ones_sb, 1.0)
            m0_sb = small.tile([128, QB], f16, name="m0")
            nc.gpsimd.dma_start(out=m0_sb, in_=m0f[:, :])
            m1_sb = small.tile([128, QB], f16, name="m1")
            nc.gpsimd.dma_start(out=m1_sb, in_=m1f[:, :])
            wo_sb = small.tile([128, ET, DV], f16, name="wo")
            nc.gpsimd.dma_start(
                out=wo_sb, in_=woT.rearrange("(t p) v -> p t v", p=128)
            )

            # ---------- projections ----------
            # x and all weights fully SBUF-resident in fp16: every tile is
            # written exactly once by DMA (no WAR hazards on input buffers).
            with (
                tc.tile_pool(name="w", bufs=1) as wp,
                tc.tile_pool(name="xin", bufs=1) as xin,
                tc.tile_pool(name="pproj", bufs=4, space="PSUM") as pproj,
            ):
                xk_sb = [xin.tile([128, S], f16, tag=f"xk{d}", name=f"xk{d}") for d in range(DT)]
                xq_sb = [xin.tile([128, NQ], f16, tag=f"xq{d}", name=f"xq{d}") for d in range(DT)]
                wk_t = [wp.tile([128, D], f16, tag=f"wk{d}", name=f"wk{d}") for d in range(DT)]
                wv_t = [wp.tile([128, D], f16, tag=f"wv{d}", name=f"wv{d}") for d in range(DT)]
                wq_t = [wp.tile([128, D], f16, tag=f"wq{d}", name=f"wq{d}") for d in range(DT)]
                # DMA order tuned for earliest PE start. Descriptor prep is
                # ~600ns each and serial per trigger engine, so the critical
                # first tiles are split across BOTH HWDGE trigger engines
                # (SP carries wk, ACT carries xk chunk 0) and interleaved.
                half = D // 2
                for d in range(DT):
                    ds = slice(d * 128, (d + 1) * 128)
                    nc.sync.dma_start(out=wk_t[d][:, :half], in_=wkT[ds, :half])
                    nc.scalar.dma_start(
                        out=xk_sb[d][:, :PCHUNK], in_=xkvT[ds, :PCHUNK]
                    )
                for d in range(DT):
                    ds = slice(d * 128, (d + 1) * 128)
                    nc.sync.dma_start(out=wk_t[d][:, half:], in_=wkT[ds, half:])
                    nc.scalar.dma_start(out=wv_t[d], in_=wvT[ds, :])
                # xk chunk 1 entirely on ACT (behind wv), chunk 2 on SP,
                # chunk 3 on ACT: each lands well before its compute window.
                for c, eng in ((1, nc.scalar), (2, nc.sync), (3, nc.scalar)):
                    cs = slice(c * PCHUNK, (c + 1) * PCHUNK)
                    for d in range(DT):
                        ds = slice(d * 128, (d + 1) * 128)
                        eng.dma_start(out=xk_sb[d][:, cs], in_=xkvT[ds, cs])
                for d in range(DT):
                    ds = slice(d * 128, (d + 1) * 128)
                    eng = nc.sync if d % 2 == 0 else nc.scalar
                    eng.dma_start(out=wq_t[d], in_=wqT[ds, :])
                for d in range(DT):
                    ds = slice(d * 128, (d + 1) * 128)
                    eng = nc.sync if d % 2 == 0 else nc.scalar
                    eng.dma_start(out=xq_sb[d], in_=xqT[ds, :])

                def kt_chunk(c):
                    cs = slice(c * PCHUNK, (c + 1) * PCHUNK)
                    for e in range(ET):
                        ps = pproj.tile([128, PCHUNK], f32, tag="pproj", name="pproj")
                        for d in range(DT):
                            nc.tensor.matmul(
                                ps,
                                lhsT=wk_t[d][:, e * 128 : (e + 1) * 128],
                                rhs=xk_sb[d][:, cs],
                                start=(d == 0),
                                stop=(d == DT - 1),
                            )
                        nc.vector.tensor_copy(kt_sb[e][:, cs], ps)

                def v_chunk(c):
                    for kt in range(PCHUNK // 128):
                        kg = c * (PCHUNK // 128) + kt
                        ks = slice(kg * 128, (kg + 1) * 128)
                        for nh in range(2):
                            ps = pproj.tile([128, PCHUNK], f32, tag="pproj", name="pproj")
                            for d in range(DT):
                                nc.tensor.matmul(
                                    ps,
                                    lhsT=xk_sb[d][:, ks],
                                    rhs=wv_t[d][:, nh * PCHUNK : (nh + 1) * PCHUNK],
                                    start=(d == 0),
                                    stop=(d == DT - 1),
                                )
                            nc.vector.tensor_copy(
                                v_sb[kg][:, nh * PCHUNK : (nh + 1) * PCHUNK], ps
                            )

                def qt_chunk(c):
                    cs = slice(c * PCHUNK, (c + 1) * PCHUNK)
                    for e in range(ET):
                        ps = pproj.tile([128, PCHUNK], f32, tag="pproj", name="pproj")
                        for d in range(DT):
                            nc.tensor.matmul(
                                ps,
                                lhsT=wq_t[d][:, e * 128 : (e + 1) * 128],
                                rhs=xq_sb[d][:, cs],
                                start=(d == 0),
                                stop=(d == DT - 1),
                            )
                        nc.vector.tensor_copy(qt_sb[e][:, cs], ps)

                for c in range(S // PCHUNK):
                    kt_chunk(c)
                    v_chunk(c)
                qt_chunk(0)
                qt_chunk(1)

            # ---------- attention ----------
            with (
                tc.tile_pool(name="pst", bufs=2, space="PSUM") as pst,
                tc.tile_pool(name="pctx", bufs=1, space="PSUM") as pctx,
                tc.tile_pool(name="psum_s", bufs=1, space="PSUM") as psums,
                tc.tile_pool(name="pout", bufs=1, space="PSUM") as pout,
            ):
                for t in range(N_QB):  # pairs: profiles (4t+2, 4t+4)
                    Pa = 4 * t + 2
                    qs = slice(t * QB, (t + 1) * QB)
                    qs1 = slice(t * QB + 128, (t + 1) * QB)
                    ctx = pctx.tile([128, ET, QB], mybir.dt.float32, tag="ctx", name="ctx")
                    sums = psums.tile([128, 2], mybir.dt.float32, tag="sums", name="sums")
                    # joint chunks: both halves of the pair attend (N=256)
                    for kc in range(Pa):
                        st = pst.tile([128, QB], mybir.dt.float32, tag="st", name="st")
                        for e in range(ET):
                            nc.tensor.matmul(
                                st,
                                lhsT=kt_sb[e][:, kc * 128 : (kc + 1) * 128],
                                rhs=qt_sb[e][:, qs],
                                start=(e == 0),
                                stop=(e == ET - 1),
                            )
                        pt = attn_sb.tile([128, QB], f16, tag="pt", name="pt")
                        nc.scalar.activation(
                            pt, st, mybir.ActivationFunctionType.Exp, scale=SCALE
                        )
                        if kc == Pa - 2:
                            nc.vector.tensor_mul(pt, pt, m0_sb)
                        elif kc == Pa - 1:
                            nc.vector.tensor_mul(pt, pt, m1_sb)
                        for h in range(2):
                            nc.tensor.matmul(
                                sums[:, h : h + 1],
                                lhsT=pt[:, h * 128 : (h + 1) * 128],
                                rhs=ones_sb,
                                start=(kc == 0 and h == 0),
                                stop=False,
                            )
                        # ctx: 2 e-slices share a PSUM bank; start zeroes the
                        # whole bank so only the first slice starts it.
                        for e in range(ET):
                            nc.tensor.matmul(
                                ctx[:, e, :],
                                lhsT=v_sb[kc][:, e * 128 : (e + 1) * 128],
                                rhs=pt,
                                start=(kc == 0 and e % 2 == 0),
                                stop=False,
                            )
                    # tail chunks: only the pair's second half attends (N=128)
                    for kc in (Pa, Pa + 1):
                        st = pst.tile([128, QB], mybir.dt.float32, tag="st", name="st")
                        for e in range(ET):
                            nc.tensor.matmul(
                                st[:, :128],
                                lhsT=kt_sb[e][:, kc * 128 : (kc + 1) * 128],
                                rhs=qt_sb[e][:, qs1],
                                start=(e == 0),
                                stop=(e == ET - 1),
                            )
                        pt = attn_sb.tile([128, QB], f16, tag="pt", name="pt")
                        nc.scalar.activation(
                            pt[:, :128],
                            st[:, :128],
                            mybir.ActivationFunctionType.Exp,
                            scale=SCALE,
                        )
                        msk = m0_sb if kc == Pa else m1_sb
                        nc.vector.tensor_mul(pt[:, :128], pt[:, :128], msk[:, :128])
                        nc.tensor.matmul(
                            sums[:, 1:2],
                            lhsT=pt[:, :128],
                            rhs=ones_sb,
                            start=False,
                            stop=(kc == Pa + 1),
                        )
                        for e in range(ET):
                            nc.tensor.matmul(
                                ctx[:, e, 128:QB],
                                lhsT=v_sb[kc][:, e * 128 : (e + 1) * 128],
                                rhs=pt[:, :128],
                                start=False,
                                stop=(kc == Pa + 1 and e % 2 == 1),
                            )
                    # epilogue for this pair
                    ctxsb = ctxsbp.tile([128, ET, QB], f16, tag="ctxsb", name="ctxsb")
                    rec = attn_sb.tile([128, 2], mybir.dt.float32, tag="rec", name="rec")
                    nc.vector.reciprocal(rec, sums)
                    for h in range(2):
                        for e in range(ET):
                            nc.vector.tensor_copy(
                                ctxsb[:, e, h * 128 : (h + 1) * 128],
                                ctx[:, e, h * 128 : (h + 1) * 128],
                            )
                        ops = pout.tile([128, DV], mybir.dt.float32, tag="ops", name="ops")
                        for e in range(ET):
                            nc.tensor.matmul(
                                ops,
                                lhsT=ctxsb[:, e, h * 128 : (h + 1) * 128],
                                rhs=wo_sb[:, e, :],
                                start=(e == 0),
                                stop=(e == ET - 1),
                            )
                        o = osbp.tile([128, DV], mybir.dt.float32, tag="o", name="o")
                        nc.scalar.activation(
                            o,
                            ops,
                            mybir.ActivationFunctionType.Copy,
                            bias=0.0,
                            scale=rec[:, h : h + 1],
                        )
                        r0 = t * QB + h * 128
                        nc.sync.dma_start(out=out[r0 : r0 + 128, :], in_=o)

    _cache["nc"] = nc
    return nc


def make_in_maps(in_features, Wq, Wk, Wv, Wo):
    x = np.asarray(in_features, dtype=np.float32).astype(np.float16)
    WqT = np.ascontiguousarray(np.asarray(Wq, dtype=np.float32).T.astype(np.float16))
    WkT = np.ascontiguousarray(np.asarray(Wk, dtype=np.float32).T.astype(np.float16))
    WvT = np.ascontiguousarray(np.asarray(Wv, dtype=np.float32).T.astype(np.float16))
    WoT = np.ascontiguousarray(np.asarray(Wo, dtype=np.float32).T.astype(np.float16))
    tri = np.tril(np.ones((128, 128), np.float16)).T  # keep iff q_local >= k_local
    onesq = np.ones((128, 128), np.float16)
    zer = np.zeros((128, 128), np.float16)
    in_maps = []
    for c in range(8):
        b, par = divmod(c, 2)
        xkvT = np.ascontiguousarray(x[b].T)
        blocks = [4 * t + 2 * h + par for t in range(4) for h in range(2)]
        xqT = np.ascontiguousarray(
            np.concatenate([x[b, 128 * j : 128 * (j + 1), :] for j in blocks]).T
        )
        if par == 0:
            m0, m1 = tri, zer
        else:
            m0, m1 = onesq, tri
        in_maps.append(
            {
                "xkvT": xkvT,
                "xqT": xqT,
                "wkT": WkT,
                "wvT": WvT,
                "wqT": WqT,
                "woT": WoT,
                "m0f": np.ascontiguousarray(np.concatenate([m0, onesq], axis=1)),
                "m1f": np.ascontiguousarray(np.concatenate([m1, onesq], axis=1)),
            }
        )
    return in_maps


def kernel(in_features, Wq, Wk, Wv, Wo, trace=False):
    from concourse.bass_utils import run_bass_kernel_spmd

    nc = _build()
    in_maps = make_in_maps(in_features, Wq, Wk, Wv, Wo)
    res = run_bass_kernel_spmd(nc, in_maps, core_ids=list(range(8)), trace=trace)
    out = np.empty((B, S, DV), np.float32)
    for c in range(8):
        b, par = divmod(c, 2)
        o = res.results[c]["out"]
        for t in range(4):
            for h in range(2):
                j = 4 * t + 2 * h + par
                r = (2 * t + h) * 128
                out[b, 128 * j : 128 * (j + 1), :] = o[r : r + 128, :]
    if trace:
        kernel.last_result = res
    return out


# revision 31
# speedup vs baseline: 1.2822x; 1.2822x over previous
"""Causal single-head attention (d_model-wide) for Trainium2, 8-core SPMD.

Problem: B=4, S=2048, D=1024; Q/K/V = x@W{q,k,v}.T; scores=Q@K.T (causal,
scale 1/sqrt(D)); out = softmax(scores)@V @ Wo.T -> [B, S, 64].

Sharding (profile-uniform causal blocks): each batch b has 16 query blocks of
128 rows; block j needs j+1 key chunks (causal). Core 2b gets the odd-count
blocks {0,2,...,14}, core 2b+1 the even-count blocks {1,3,...,15}; counts
round up to the even profile P in {2,4,...,16}, so EVERY core runs the same
chunk schedule (one SPMD program) while skipping all score blocks above the
causal diagonal. Blocks pair as (P, P+2) into 256-wide matmul tiles: chunks
< P serve both halves (N=256), the last two chunks serve only the second
half (N=128). The only per-core differences are data: which x rows feed Q,
and two [128,256] fp16 multiplicative masks m0/m1 (triangle/ones/zeros)
applied post-exp at each block's last two chunks.

On-device layout (no transposes anywhere; host pre-transposes x and W):
  KT[e, k] = sum_d WkT[d,e] xT[d,k]      (lhsT=WkT tile, rhs=xT tile)
  QT[e, q] likewise; V[k, e] = sum_d xT[d,k] WvT[d,e] (lhsT=xT, rhs=WvT)
  ST[k, q] = sum_e KT[e,k] QT[e,q]       (scores, transposed; PSUM f32)
  PT[k, q] = exp(ST/32) * mask           (ACT exp, fp16; no max-subtraction:
                                          |ST|/32 <~ 8 so exp is fp32-safe)
  ctxT[e, q] += sum_k V[k,e] PT[k,q]     (PSUM f32 over k chunks; 2 e-slices
                                          per 2KB bank: one start/stop per
                                          bank since start zeroes the bank)
  sums[q]   += sum_k PT[k,q]             (ones-matmul into PSUM)
  out[q, v] = (sum_e ctxT[e,q] WoT[e,v]) * (1/sums[q])

Everything is fp16 storage (inputs converted on host) with fp32 PSUM
accumulation; measured rel err vs the fp32 reference is ~5e-4.
"""

import numpy as np

B, S, D, DV = 4, 2048, 1024, 64
NQ = S // 2          # queries per core
ET = D // 128        # 8 e-tiles
DT = D // 128        # 8 d-tiles
KT_N = S // 128      # 16 key tiles of 128
PCHUNK = 512         # projection free-dim chunk
QB = 256             # query block (free dim of attention matmuls)
N_QB = NQ // QB      # 4
SCALE = 1.0 / 32.0   # 1/sqrt(D)
NEG = -1.0e30

_cache = {}


def _patch_drain_split(tile_mod, mybir, ScopedClock):
    """This walrus build accepts only ONE sync-wait on the kernel-tail SP
    Drain; split the waits across multiple drain instructions."""
    if getattr(tile_mod.TileContext, "_drain_split_patched", False):
        return

    def _patched(self, tick_clock, wait_clock):
        nc = self.nc
        drain_inst = nc.sync.drain()
        wait_clock.add_sem_waits(
            drain_inst.ins, ScopedClock({None: tick_clock.global_clock})
        )
        ins = drain_inst.ins
        waits = list(ins.sync_info.on_wait or [])
        if len(waits) > 1:
            ins.sync_info = mybir.SyncInfo(
                on_wait=waits[:1], on_update=list(ins.sync_info.on_update or [])
            )
            for w in waits[1:]:
                extra = nc.sync.drain()
                extra.ins.sync_info = mybir.SyncInfo(on_wait=[w], on_update=[])
        nc.all_engine_barrier()
        assert self.sems is not None
        popped = nc._tile_sem_poison_stack.pop()
        assert popped is self._sem_poison
        nc.clear_and_free_semaphores(list(self.sems.allocated().values()))
        nc.all_engine_barrier()

    tile_mod.TileContext._drain_and_barrier = _patched
    tile_mod.TileContext._drain_split_patched = True


def _split_multiwait_bir(bir_json):
    """This walrus build accepts only one sync-wait per instruction. Rewrite
    the BIR so any instruction with N>1 waits is preceded by N-1 single-wait
    Drain instructions on the same engine (engine streams execute in block
    order, so the waits are enforced before the instruction issues)."""
    import json

    m = json.loads(bir_json)
    changed = False
    for fn in m.get("functions", []):
        for blk in fn.get("blocks", []):
            insts = blk.get("instructions", [])
            out = []
            for inst in insts:
                si = inst.get("sync_info")
                waits = (si or {}).get("on_wait") or []
                if len(waits) > 1:
                    changed = True
                    for i, w in enumerate(waits[:-1]):
                        out.append(
                            {
                                "name": f"{inst['name']}-sw{i}",
                                "opcode": "Drain",
                                "engine": inst["engine"],
                                "ins": [],
                                "outs": [],
                                "is_reset_sema": False,
                                "sync_info": {"on_wait": [w], "on_update": []},
                                "debug": inst.get("debug"),
                            }
                        )
                    si["on_wait"] = [waits[-1]]
                out.append(inst)
            blk["instructions"] = out
    if not changed:
        return bir_json
    return json.dumps(m).encode()


def _patch_compile_hook():
    """Route every BIR compile through _split_multiwait_bir."""
    import concourse.bass_utils as bu

    if getattr(bu, "_multiwait_patched", False):
        return
    orig = bu.compile_bir_kernel

    def wrapped(bir_json, tmpdir, neff_name="file.neff"):
        return orig(_split_multiwait_bir(bir_json), tmpdir, neff_name)

    bu.compile_bir_kernel = wrapped
    bu._multiwait_patched = True
    try:
        import concourse.bass2jax as b2j

        b2j.compile_bir_kernel = wrapped
    except Exception:
        pass


def _install_ntff_hook():
    """Provide antenv.axon_hooks (absent in this image) so that
    run_bass_kernel_spmd(trace=True) can profile through axon."""
    import sys
    import types

    if "antenv.axon_hooks" in sys.modules:
        return
    try:
        import trn_agent_boot.trn_boot as tb

        mod = types.ModuleType("antenv.axon_hooks")
        _hook = [None]
        mod.set_axon_ntff_profile_hook = lambda h: _hook.__setitem__(0, h)
        mod.get_axon_ntff_profile_hook = lambda: _hook[0]
        sys.modules["antenv.axon_hooks"] = mod
        mod.set_axon_ntff_profile_hook(
            tb._ntff_profile_via_ctypes("/opt/axon/libaxon_pjrt.so")
        )
    except Exception:
        pass


def _build():
    if "nc" in _cache:
        return _cache["nc"]

    import concourse.bass as bass
    import concourse.mybir as mybir
    import concourse.tile as tile
    from concourse.vector_clock import ScopedClock

    _patch_drain_split(tile, mybir, ScopedClock)
    _patch_compile_hook()
    _install_ntff_hook()

    f32 = mybir.dt.float32
    f32r = mybir.dt.float32r
    f16 = mybir.dt.float16

    nc = bass.Bass()
    xkvT = nc.dram_tensor("xkvT", [D, S], f16, kind="ExternalInput")
    xqT = nc.dram_tensor("xqT", [D, NQ], f16, kind="ExternalInput")
    wkT = nc.dram_tensor("wkT", [D, D], f16, kind="ExternalInput")
    xkv = nc.dram_tensor("xkv", [S, D], f16, kind="ExternalInput")
    wqT = nc.dram_tensor("wqT", [D, D], f16, kind="ExternalInput")
    woT = nc.dram_tensor("woT", [D, DV], f16, kind="ExternalInput")
    m0f = nc.dram_tensor("m0f", [128, QB], f16, kind="ExternalInput")
    m1f = nc.dram_tensor("m1f", [128, QB], f16, kind="ExternalInput")
    out = nc.dram_tensor("out", [NQ, DV], f32, kind="ExternalOutput")

    with tile.TileContext(nc) as tc:
        with (
            tc.tile_pool(name="kt", bufs=1) as ktp,
            tc.tile_pool(name="v", bufs=1) as vp,
            tc.tile_pool(name="qt", bufs=1) as qtp,
            tc.tile_pool(name="small", bufs=1) as small,
            tc.tile_pool(name="attn_sb", bufs=4) as attn_sb,
            tc.tile_pool(name="ctxsb", bufs=2) as ctxsbp,
            tc.tile_pool(name="osb", bufs=2) as osbp,
        ):
            # persistent fp16 operand stores
            kt_sb = [ktp.tile([128, S], f16, tag=f"kt{e}", name=f"kt{e}") for e in range(ET)]
            v_sb = [vp.tile([128, D], f16, tag=f"v{k}", name=f"v{k}") for k in range(KT_N)]
            qt_sb = [qtp.tile([128, NQ], f16, tag=f"qt{e}", name=f"qt{e}") for e in range(ET)]

            ones_sb = small.tile([128, 1], f16, name="ones")
            nc.vector.memset(ones_sb, 1.0)
            m0_sb = small.tile([128, QB], f16, name="m0")
            nc.gpsimd.dma_start(out=m0_sb, in_=m0f[:, :])
            m1_sb = small.tile([128, QB], f16, name="m1")
            nc.gpsimd.dma_start(out=m1_sb, in_=m1f[:, :])
            wo_sb = small.tile([128, ET, DV], f16, name="wo")
            nc.gpsimd.dma_start(
                out=wo_sb, in_=woT.rearrange("(t p) v -> p t v", p=128)
            )

            # ---------- projections ----------
            # x and all weights fully SBUF-resident in fp16: every tile is
            # written exactly once by DMA (no WAR hazards on input buffers).
            with (
                tc.tile_pool(name="w", bufs=1) as wp,
                tc.tile_pool(name="xin", bufs=1) as xin,
                tc.tile_pool(name="pproj", bufs=4, space="PSUM") as pproj,
            ):
                xk_sb = [xin.tile([128, S], f16, tag=f"xk{d}", name=f"xk{d}") for d in range(DT)]
                xq_sb = [xin.tile([128, NQ], f16, tag=f"xq{d}", name=f"xq{d}") for d in range(DT)]
                wk_t = [wp.tile([128, D], f16, tag=f"wk{d}", name=f"wk{d}") for d in range(DT)]
                wq_t = [wp.tile([128, D], f16, tag=f"wq{d}", name=f"wq{d}") for d in range(DT)]
                # DMA order tuned for earliest PE start. Descriptor prep is
                # ~600ns each and serial per trigger engine, so the critical
                # first tiles are split across BOTH HWDGE trigger engines
                # (SP carries wk, ACT carries xk chunk 0) and interleaved.
                half = D // 2
                for d in range(DT):
                    ds = slice(d * 128, (d + 1) * 128)
                    nc.sync.dma_start(out=wk_t[d][:, :half], in_=wkT[ds, :half])
                    nc.scalar.dma_start(
                        out=xk_sb[d][:, :PCHUNK], in_=xkvT[ds, :PCHUNK]
                    )
                for d in range(DT):
                    ds = slice(d * 128, (d + 1) * 128)
                    nc.sync.dma_start(out=wk_t[d][:, half:], in_=wkT[ds, half:])
                # xk chunk 1 entirely on ACT (behind wv), chunk 2 on SP,
                # chunk 3 on ACT: each lands well before its compute window.
                for c, eng in ((1, nc.scalar), (2, nc.sync), (3, nc.scalar)):
                    cs = slice(c * PCHUNK, (c + 1) * PCHUNK)
                    for d in range(DT):
                        ds = slice(d * 128, (d + 1) * 128)
                        eng.dma_start(out=xk_sb[d][:, cs], in_=xkvT[ds, cs])
                for d in range(DT):
                    ds = slice(d * 128, (d + 1) * 128)
                    eng = nc.sync if d % 2 == 0 else nc.scalar
                    eng.dma_start(out=wq_t[d], in_=wqT[ds, :])
                for d in range(DT):
                    ds = slice(d * 128, (d + 1) * 128)
                    eng = nc.sync if d % 2 == 0 else nc.scalar
                    eng.dma_start(out=xq_sb[d], in_=xqT[ds, :])
                for kg in range(KT_N):
                    eng = nc.sync if kg % 2 == 0 else nc.scalar
                    eng.dma_start(
                        out=v_sb[kg], in_=xkv[kg * 128 : (kg + 1) * 128, :]
                    )

                def kt_chunk(c):
                    cs = slice(c * PCHUNK, (c + 1) * PCHUNK)
                    for e in range(ET):
                        ps = pproj.tile([128, PCHUNK], f32, tag="pproj", name="pproj")
                        for d in range(DT):
                            nc.tensor.matmul(
                                ps,
                                lhsT=wk_t[d][:, e * 128 : (e + 1) * 128],
                                rhs=xk_sb[d][:, cs],
                                start=(d == 0),
                                stop=(d == DT - 1),
                            )
                        nc.vector.tensor_copy(kt_sb[e][:, cs], ps)

                def qt_chunk(c):
                    cs = slice(c * PCHUNK, (c + 1) * PCHUNK)
                    for e in range(ET):
                        ps = pproj.tile([128, PCHUNK], f32, tag="pproj", name="pproj")
                        for d in range(DT):
                            nc.tensor.matmul(
                                ps,
                                lhsT=wq_t[d][:, e * 128 : (e + 1) * 128],
                                rhs=xq_sb[d][:, cs],
                                start=(d == 0),
                                stop=(d == DT - 1),
                            )
                        nc.vector.tensor_copy(qt_sb[e][:, cs], ps)

                for c in range(S // PCHUNK):
                    kt_chunk(c)
                qt_chunk(0)
                qt_chunk(1)

            # ---------- attention ----------
            with (
                tc.tile_pool(name="pst", bufs=2, space="PSUM") as pst,
                tc.tile_pool(name="pctx", bufs=1, space="PSUM") as pctx,
                tc.tile_pool(name="psum_s", bufs=1, space="PSUM") as psums,
                tc.tile_pool(name="pout", bufs=1, space="PSUM") as pout,
            ):
                for t in range(N_QB):  # pairs: profiles (4t+2, 4t+4)
                    Pa = 4 * t + 2
                    qs = slice(t * QB, (t + 1) * QB)
                    qs1 = slice(t * QB + 128, (t + 1) * QB)
                    ctx = pctx.tile([128, ET, QB], mybir.dt.float32, tag="ctx", name="ctx")
                    sums = psums.tile([128, 2], mybir.dt.float32, tag="sums", name="sums")
                    # joint chunks: both halves of the pair attend (N=256)
                    for kc in range(Pa):
                        st = pst.tile([128, QB], mybir.dt.float32, tag="st", name="st")
                        for e in range(ET):
                            nc.tensor.matmul(
                                st,
                                lhsT=kt_sb[e][:, kc * 128 : (kc + 1) * 128],
                                rhs=qt_sb[e][:, qs],
                                start=(e == 0),
                                stop=(e == ET - 1),
                            )
                        pt = attn_sb.tile([128, QB], f16, tag="pt", name="pt")
                        nc.scalar.activation(
                            pt, st, mybir.ActivationFunctionType.Exp, scale=SCALE
                        )
                        if kc == Pa - 2:
                            nc.vector.tensor_mul(pt, pt, m0_sb)
                        elif kc == Pa - 1:
                            nc.vector.tensor_mul(pt, pt, m1_sb)
                        for h in range(2):
                            nc.tensor.matmul(
                                sums[:, h : h + 1],
                                lhsT=pt[:, h * 128 : (h + 1) * 128],
                                rhs=ones_sb,
                                start=(kc == 0 and h == 0),
                                stop=False,
                            )
                        # ctx: 2 e-slices share a PSUM bank; start zeroes the
                        # whole bank so only the first slice starts it.
                        for e in range(ET):
                            nc.tensor.matmul(
                                ctx[:, e, :],
                                lhsT=v_sb[kc][:, e * 128 : (e + 1) * 128],
                                rhs=pt,
                                start=(kc == 0 and e % 2 == 0),
                                stop=False,
                            )
                    # tail chunks: only the pair's second half attends (N=128)
                    for kc in (Pa, Pa + 1):
                        st = pst.tile([128, QB], mybir.dt.float32, tag="st", name="st")
                        for e in range(ET):
                            nc.tensor.matmul(
                                st[:, :128],
                                lhsT=kt_sb[e][:, kc * 128 : (kc + 1) * 128],
                                rhs=qt_sb[e][:, qs1],
                                start=(e == 0),
                                stop=(e == ET - 1),
                            )
                        pt = attn_sb.tile([128, QB], f16, tag="pt", name="pt")
                        nc.scalar.activation(
                            pt[:, :128],
                            st[:, :128],
                            mybir.ActivationFunctionType.Exp,
                            scale=SCALE,
                        )
                        msk = m0_sb if kc == Pa else m1_sb
                        nc.vector.tensor_mul(pt[:, :128], pt[:, :128], msk[:, :128])
                        nc.tensor.matmul(
                            sums[:, 1:2],
                            lhsT=pt[:, :128],
                            rhs=ones_sb,
                            start=False,
                            stop=(kc == Pa + 1),
                        )
                        for e in range(ET):
                            nc.tensor.matmul(
                                ctx[:, e, 128:QB],
                                lhsT=v_sb[kc][:, e * 128 : (e + 1) * 128],
                                rhs=pt[:, :128],
                                start=False,
                                stop=(kc == Pa + 1 and e % 2 == 1),
                            )
                    # epilogue for this pair
                    ctxsb = ctxsbp.tile([128, ET, QB], f16, tag="ctxsb", name="ctxsb")
                    rec = attn_sb.tile([128, 2], mybir.dt.float32, tag="rec", name="rec")
                    nc.vector.reciprocal(rec, sums)
                    for h in range(2):
                        for e in range(ET):
                            nc.vector.tensor_copy(
                                ctxsb[:, e, h * 128 : (h + 1) * 128],
                                ctx[:, e, h * 128 : (h + 1) * 128],
                            )
                        ops = pout.tile([128, DV], mybir.dt.float32, tag="ops", name="ops")
                        for e in range(ET):
                            nc.tensor.matmul(
                                ops,
                                lhsT=ctxsb[:, e, h * 128 : (h + 1) * 128],
                                rhs=wo_sb[:, e, :],
                                start=(e == 0),
                                stop=(e == ET - 1),
                            )
                        o = osbp.tile([128, DV], mybir.dt.float32, tag="o", name="o")
                        nc.scalar.activation(
                            o,
                            ops,
                            mybir.ActivationFunctionType.Copy,
                            bias=0.0,
                            scale=rec[:, h : h + 1],
                        )
                        r0 = t * QB + h * 128
                        nc.sync.dma_start(out=out[r0 : r0 + 128, :], in_=o)

    _cache["nc"] = nc
    return nc


def make_in_maps(in_features, Wq, Wk, Wv, Wo):
    x = np.asarray(in_features, dtype=np.float32).astype(np.float16)
    WqT = np.ascontiguousarray(np.asarray(Wq, dtype=np.float32).T.astype(np.float16))
    WkT = np.ascontiguousarray(np.asarray(Wk, dtype=np.float32).T.astype(np.float16))
    # V and output projections fused: out = P@(x@Wv.T)@Wo.T = (P@x)@M
    M = np.ascontiguousarray(
        (np.asarray(Wv, np.float32).T @ np.asarray(Wo, np.float32).T).astype(
            np.float16
        )
    )
    tri = np.tril(np.ones((128, 128), np.float16)).T  # keep iff q_local >= k_local
    onesq = np.ones((128, 128), np.float16)
    zer = np.zeros((128, 128), np.float16)
    in_maps = []
    for c in range(8):
        b, par = divmod(c, 2)
        xkvT = np.ascontiguousarray(x[b].T)
        blocks = [4 * t + 2 * h + par for t in range(4) for h in range(2)]
        xqT = np.ascontiguousarray(
            np.concatenate([x[b, 128 * j : 128 * (j + 1), :] for j in blocks]).T
        )
        if par == 0:
            m0, m1 = tri, zer
        else:
            m0, m1 = onesq, tri
        in_maps.append(
            {
                "xkvT": xkvT,
                "xqT": xqT,
                "wkT": WkT,
                "xkv": np.ascontiguousarray(x[b]),
                "wqT": WqT,
                "woT": M,
                "m0f": np.ascontiguousarray(np.concatenate([m0, onesq], axis=1)),
                "m1f": np.ascontiguousarray(np.concatenate([m1, onesq], axis=1)),
            }
        )
    return in_maps


def kernel(in_features, Wq, Wk, Wv, Wo, trace=False):
    from concourse.bass_utils import run_bass_kernel_spmd

    nc = _build()
    in_maps = make_in_maps(in_features, Wq, Wk, Wv, Wo)
    res = run_bass_kernel_spmd(nc, in_maps, core_ids=list(range(8)), trace=trace)
    out = np.empty((B, S, DV), np.float32)
    for c in range(8):
        b, par = divmod(c, 2)
        o = res.results[c]["out"]
        for t in range(4):
            for h in range(2):
                j = 4 * t + 2 * h + par
                r = (2 * t + h) * 128
                out[b, 128 * j : 128 * (j + 1), :] = o[r : r + 128, :]
    if trace:
        kernel.last_result = res
    return out


# revision 32
# speedup vs baseline: 1.5519x; 1.2104x over previous
"""Causal single-head attention (d_model-wide) for Trainium2, 8-core SPMD.

Problem: B=4, S=2048, D=1024; Q/K/V = x@W{q,k,v}.T; scores=Q@K.T (causal,
scale 1/sqrt(D)); out = softmax(scores)@V @ Wo.T -> [B, S, 64].

Sharding (profile-uniform causal blocks): each batch b has 16 query blocks of
128 rows; block j needs j+1 key chunks (causal). Core 2b gets the odd-count
blocks {0,2,...,14}, core 2b+1 the even-count blocks {1,3,...,15}; counts
round up to the even profile P in {2,4,...,16}, so EVERY core runs the same
chunk schedule (one SPMD program) while skipping all score blocks above the
causal diagonal. Blocks pair as (P, P+2) into 256-wide matmul tiles: chunks
< P serve both halves (N=256), the last two chunks serve only the second
half (N=128). The only per-core differences are data: which x rows feed Q,
and two [128,256] fp16 multiplicative masks m0/m1 (triangle/ones/zeros)
applied post-exp at each block's last two chunks.

On-device layout (no transposes anywhere; host pre-transposes x and W):
  KT[e, k] = sum_d WkT[d,e] xT[d,k]      (lhsT=WkT tile, rhs=xT tile)
  QT[e, q] likewise; V[k, e] = sum_d xT[d,k] WvT[d,e] (lhsT=xT, rhs=WvT)
  ST[k, q] = sum_e KT[e,k] QT[e,q]       (scores, transposed; PSUM f32)
  PT[k, q] = exp(ST/32) * mask           (ACT exp, fp16; no max-subtraction:
                                          |ST|/32 <~ 8 so exp is fp32-safe)
  ctxT[e, q] += sum_k V[k,e] PT[k,q]     (PSUM f32 over k chunks; 2 e-slices
                                          per 2KB bank: one start/stop per
                                          bank since start zeroes the bank)
  sums[q]   += sum_k PT[k,q]             (ones-matmul into PSUM)
  out[q, v] = (sum_e ctxT[e,q] WoT[e,v]) * (1/sums[q])

Everything is fp16 storage (inputs converted on host) with fp32 PSUM
accumulation; measured rel err vs the fp32 reference is ~5e-4.
"""

import numpy as np

B, S, D, DV = 4, 2048, 1024, 64
NQ = S // 2          # queries per core
ET = D // 128        # 8 e-tiles
DT = D // 128        # 8 d-tiles
KT_N = S // 128      # 16 key tiles of 128
PCHUNK = 512         # projection free-dim chunk
QB = 256             # query block (free dim of attention matmuls)
N_QB = NQ // QB      # 4
SCALE = 1.0 / 32.0   # 1/sqrt(D)
NEG = -1.0e30

_cache = {}


def _patch_drain_split(tile_mod, mybir, ScopedClock):
    """This walrus build accepts only ONE sync-wait on the kernel-tail SP
    Drain; split the waits across multiple drain instructions."""
    if getattr(tile_mod.TileContext, "_drain_split_patched", False):
        return

    def _patched(self, tick_clock, wait_clock):
        nc = self.nc
        drain_inst = nc.sync.drain()
        wait_clock.add_sem_waits(
            drain_inst.ins, ScopedClock({None: tick_clock.global_clock})
        )
        ins = drain_inst.ins
        waits = list(ins.sync_info.on_wait or [])
        if len(waits) > 1:
            ins.sync_info = mybir.SyncInfo(
                on_wait=waits[:1], on_update=list(ins.sync_info.on_update or [])
            )
            for w in waits[1:]:
                extra = nc.sync.drain()
                extra.ins.sync_info = mybir.SyncInfo(on_wait=[w], on_update=[])
        nc.all_engine_barrier()
        assert self.sems is not None
        popped = nc._tile_sem_poison_stack.pop()
        assert popped is self._sem_poison
        nc.clear_and_free_semaphores(list(self.sems.allocated().values()))
        nc.all_engine_barrier()

    tile_mod.TileContext._drain_and_barrier = _patched
    tile_mod.TileContext._drain_split_patched = True


def _split_multiwait_bir(bir_json):
    """This walrus build accepts only one sync-wait per instruction. Rewrite
    the BIR so any instruction with N>1 waits is preceded by N-1 single-wait
    Drain instructions on the same engine (engine streams execute in block
    order, so the waits are enforced before the instruction issues)."""
    import json

    m = json.loads(bir_json)
    changed = False
    for fn in m.get("functions", []):
        for blk in fn.get("blocks", []):
            insts = blk.get("instructions", [])
            out = []
            for inst in insts:
                si = inst.get("sync_info")
                waits = (si or {}).get("on_wait") or []
                if len(waits) > 1:
                    changed = True
                    for i, w in enumerate(waits[:-1]):
                        out.append(
                            {
                                "name": f"{inst['name']}-sw{i}",
                                "opcode": "Drain",
                                "engine": inst["engine"],
                                "ins": [],
                                "outs": [],
                                "is_reset_sema": False,
                                "sync_info": {"on_wait": [w], "on_update": []},
                                "debug": inst.get("debug"),
                            }
                        )
                    si["on_wait"] = [waits[-1]]
                out.append(inst)
            blk["instructions"] = out
    if not changed:
        return bir_json
    return json.dumps(m).encode()


def _patch_compile_hook():
    """Route every BIR compile through _split_multiwait_bir."""
    import concourse.bass_utils as bu

    if getattr(bu, "_multiwait_patched", False):
        return
    orig = bu.compile_bir_kernel

    def wrapped(bir_json, tmpdir, neff_name="file.neff"):
        return orig(_split_multiwait_bir(bir_json), tmpdir, neff_name)

    bu.compile_bir_kernel = wrapped
    bu._multiwait_patched = True
    try:
        import concourse.bass2jax as b2j

        b2j.compile_bir_kernel = wrapped
    except Exception:
        pass


def _install_ntff_hook():
    """Provide antenv.axon_hooks (absent in this image) so that
    run_bass_kernel_spmd(trace=True) can profile through axon."""
    import sys
    import types

    if "antenv.axon_hooks" in sys.modules:
        return
    try:
        import trn_agent_boot.trn_boot as tb

        mod = types.ModuleType("antenv.axon_hooks")
        _hook = [None]
        mod.set_axon_ntff_profile_hook = lambda h: _hook.__setitem__(0, h)
        mod.get_axon_ntff_profile_hook = lambda: _hook[0]
        sys.modules["antenv.axon_hooks"] = mod
        mod.set_axon_ntff_profile_hook(
            tb._ntff_profile_via_ctypes("/opt/axon/libaxon_pjrt.so")
        )
    except Exception:
        pass


def _build():
    if "nc" in _cache:
        return _cache["nc"]

    import concourse.bass as bass
    import concourse.mybir as mybir
    import concourse.tile as tile
    from concourse.vector_clock import ScopedClock

    _patch_drain_split(tile, mybir, ScopedClock)
    _patch_compile_hook()
    _install_ntff_hook()

    f32 = mybir.dt.float32
    f32r = mybir.dt.float32r
    f16 = mybir.dt.float16

    nc = bass.Bass()
    xkvT = nc.dram_tensor("xkvT", [D, S], f16, kind="ExternalInput")
    xqT = nc.dram_tensor("xqT", [D, NQ], f16, kind="ExternalInput")
    xkv = nc.dram_tensor("xkv", [S, D], f16, kind="ExternalInput")
    wqT = nc.dram_tensor("wqT", [D, D], f16, kind="ExternalInput")
    woT = nc.dram_tensor("woT", [D, DV], f16, kind="ExternalInput")
    m0f = nc.dram_tensor("m0f", [128, QB], f16, kind="ExternalInput")
    m1f = nc.dram_tensor("m1f", [128, QB], f16, kind="ExternalInput")
    out = nc.dram_tensor("out", [NQ, DV], f32, kind="ExternalOutput")

    with tile.TileContext(nc) as tc:
        with (
            tc.tile_pool(name="kt", bufs=1) as ktp,
            tc.tile_pool(name="v", bufs=1) as vp,
            tc.tile_pool(name="qt", bufs=1) as qtp,
            tc.tile_pool(name="small", bufs=1) as small,
            tc.tile_pool(name="attn_sb", bufs=4) as attn_sb,
            tc.tile_pool(name="ctxsb", bufs=2) as ctxsbp,
            tc.tile_pool(name="osb", bufs=2) as osbp,
        ):
            # persistent fp16 operand stores
            v_sb = [vp.tile([128, D], f16, tag=f"v{k}", name=f"v{k}") for k in range(KT_N)]
            qt_sb = [qtp.tile([128, NQ], f16, tag=f"qt{e}", name=f"qt{e}") for e in range(ET)]

            xk_sb = [ktp.tile([128, S], f16, tag=f"xk{d}", name=f"xk{d}") for d in range(DT)]
            ones_sb = small.tile([128, 1], f16, name="ones")
            nc.vector.memset(ones_sb, 1.0)
            m0_sb = small.tile([128, QB], f16, name="m0")
            nc.gpsimd.dma_start(out=m0_sb, in_=m0f[:, :])
            m1_sb = small.tile([128, QB], f16, name="m1")
            nc.gpsimd.dma_start(out=m1_sb, in_=m1f[:, :])
            wo_sb = small.tile([128, ET, DV], f16, name="wo")
            nc.gpsimd.dma_start(
                out=wo_sb, in_=woT.rearrange("(t p) v -> p t v", p=128)
            )

            # ---------- projections ----------
            # x and all weights fully SBUF-resident in fp16: every tile is
            # written exactly once by DMA (no WAR hazards on input buffers).
            with (
                tc.tile_pool(name="w", bufs=1) as wp,
                tc.tile_pool(name="xin", bufs=1) as xin,
                tc.tile_pool(name="pproj", bufs=4, space="PSUM") as pproj,
            ):
                xq_sb = [xin.tile([128, NQ], f16, tag=f"xq{d}", name=f"xq{d}") for d in range(DT)]
                wq_t = [wp.tile([128, D], f16, tag=f"wq{d}", name=f"wq{d}") for d in range(DT)]
                # DMA order tuned for earliest PE start. Descriptor prep is
                # ~600ns each and serial per trigger engine, so the critical
                # first tiles are split across BOTH HWDGE trigger engines
                # (SP carries wk, ACT carries xk chunk 0) and interleaved.
                half = D // 2
                for d in range(DT):
                    ds = slice(d * 128, (d + 1) * 128)
                    eng = nc.sync if d % 2 == 0 else nc.scalar
                    eng.dma_start(out=xk_sb[d][:, :PCHUNK], in_=xkvT[ds, :PCHUNK])
                # xk chunk 1 entirely on ACT (behind wv), chunk 2 on SP,
                # chunk 3 on ACT: each lands well before its compute window.
                for c, eng in ((1, nc.scalar), (2, nc.sync), (3, nc.scalar)):
                    cs = slice(c * PCHUNK, (c + 1) * PCHUNK)
                    for d in range(DT):
                        ds = slice(d * 128, (d + 1) * 128)
                        eng.dma_start(out=xk_sb[d][:, cs], in_=xkvT[ds, cs])
                for d in range(DT):
                    ds = slice(d * 128, (d + 1) * 128)
                    eng = nc.sync if d % 2 == 0 else nc.scalar
                    eng.dma_start(out=wq_t[d], in_=wqT[ds, :])
                for d in range(DT):
                    ds = slice(d * 128, (d + 1) * 128)
                    eng = nc.sync if d % 2 == 0 else nc.scalar
                    eng.dma_start(out=xq_sb[d], in_=xqT[ds, :])
                for kg in range(KT_N):
                    eng = nc.sync if kg % 2 == 0 else nc.scalar
                    eng.dma_start(
                        out=v_sb[kg], in_=xkv[kg * 128 : (kg + 1) * 128, :]
                    )

                def qt_chunk(c):
                    cs = slice(c * PCHUNK, (c + 1) * PCHUNK)
                    for e in range(ET):
                        ps = pproj.tile([128, PCHUNK], f32, tag="pproj", name="pproj")
                        for d in range(DT):
                            nc.tensor.matmul(
                                ps,
                                lhsT=wq_t[d][:, e * 128 : (e + 1) * 128],
                                rhs=xq_sb[d][:, cs],
                                start=(d == 0),
                                stop=(d == DT - 1),
                            )
                        nc.vector.tensor_copy(qt_sb[e][:, cs], ps)

                qt_chunk(0)
                qt_chunk(1)

            # ---------- attention ----------
            with (
                tc.tile_pool(name="pst", bufs=2, space="PSUM") as pst,
                tc.tile_pool(name="pctx", bufs=1, space="PSUM") as pctx,
                tc.tile_pool(name="psum_s", bufs=1, space="PSUM") as psums,
                tc.tile_pool(name="pout", bufs=1, space="PSUM") as pout,
            ):
                for t in range(N_QB):  # pairs: profiles (4t+2, 4t+4)
                    Pa = 4 * t + 2
                    qs = slice(t * QB, (t + 1) * QB)
                    qs1 = slice(t * QB + 128, (t + 1) * QB)
                    ctx = pctx.tile([128, ET, QB], mybir.dt.float32, tag="ctx", name="ctx")
                    sums = psums.tile([128, 2], mybir.dt.float32, tag="sums", name="sums")
                    # joint chunks: both halves of the pair attend (N=256)
                    for kc in range(Pa):
                        st = pst.tile([128, QB], mybir.dt.float32, tag="st", name="st")
                        for e in range(ET):
                            nc.tensor.matmul(
                                st,
                                lhsT=xk_sb[e][:, kc * 128 : (kc + 1) * 128],
                                rhs=qt_sb[e][:, qs],
                                start=(e == 0),
                                stop=(e == ET - 1),
                            )
                        pt = attn_sb.tile([128, QB], f16, tag="pt", name="pt")
                        nc.scalar.activation(
                            pt, st, mybir.ActivationFunctionType.Exp, scale=SCALE
                        )
                        if kc == Pa - 2:
                            nc.vector.tensor_mul(pt, pt, m0_sb)
                        elif kc == Pa - 1:
                            nc.vector.tensor_mul(pt, pt, m1_sb)
                        for h in range(2):
                            nc.tensor.matmul(
                                sums[:, h : h + 1],
                                lhsT=pt[:, h * 128 : (h + 1) * 128],
                                rhs=ones_sb,
                                start=(kc == 0 and h == 0),
                                stop=False,
                            )
                        # ctx: 2 e-slices share a PSUM bank; start zeroes the
                        # whole bank so only the first slice starts it.
                        for e in range(ET):
                            nc.tensor.matmul(
                                ctx[:, e, :],
                                lhsT=v_sb[kc][:, e * 128 : (e + 1) * 128],
                                rhs=pt,
                                start=(kc == 0 and e % 2 == 0),
                                stop=False,
                            )
                    # tail chunks: only the pair's second half attends (N=128)
                    for kc in (Pa, Pa + 1):
                        st = pst.tile([128, QB], mybir.dt.float32, tag="st", name="st")
                        for e in range(ET):
                            nc.tensor.matmul(
                                st[:, :128],
                                lhsT=xk_sb[e][:, kc * 128 : (kc + 1) * 128],
                                rhs=qt_sb[e][:, qs1],
                                start=(e == 0),
                                stop=(e == ET - 1),
                            )
                        pt = attn_sb.tile([128, QB], f16, tag="pt", name="pt")
                        nc.scalar.activation(
                            pt[:, :128],
                            st[:, :128],
                            mybir.ActivationFunctionType.Exp,
                            scale=SCALE,
                        )
                        msk = m0_sb if kc == Pa else m1_sb
                        nc.vector.tensor_mul(pt[:, :128], pt[:, :128], msk[:, :128])
                        nc.tensor.matmul(
                            sums[:, 1:2],
                            lhsT=pt[:, :128],
                            rhs=ones_sb,
                            start=False,
                            stop=(kc == Pa + 1),
                        )
                        for e in range(ET):
                            nc.tensor.matmul(
                                ctx[:, e, 128:QB],
                                lhsT=v_sb[kc][:, e * 128 : (e + 1) * 128],
                                rhs=pt[:, :128],
                                start=False,
                                stop=(kc == Pa + 1 and e % 2 == 1),
                            )
                    # epilogue for this pair
                    ctxsb = ctxsbp.tile([128, ET, QB], f16, tag="ctxsb", name="ctxsb")
                    rec = attn_sb.tile([128, 2], mybir.dt.float32, tag="rec", name="rec")
                    nc.vector.reciprocal(rec, sums)
                    for h in range(2):
                        for e in range(ET):
                            nc.vector.tensor_copy(
                                ctxsb[:, e, h * 128 : (h + 1) * 128],
                                ctx[:, e, h * 128 : (h + 1) * 128],
                            )
                        ops = pout.tile([128, DV], mybir.dt.float32, tag="ops", name="ops")
                        for e in range(ET):
                            nc.tensor.matmul(
                                ops,
                                lhsT=ctxsb[:, e, h * 128 : (h + 1) * 128],
                                rhs=wo_sb[:, e, :],
                                start=(e == 0),
                                stop=(e == ET - 1),
                            )
                        o = osbp.tile([128, DV], mybir.dt.float32, tag="o", name="o")
                        nc.scalar.activation(
                            o,
                            ops,
                            mybir.ActivationFunctionType.Copy,
                            bias=0.0,
                            scale=rec[:, h : h + 1],
                        )
                        r0 = t * QB + h * 128
                        nc.sync.dma_start(out=out[r0 : r0 + 128, :], in_=o)

    _cache["nc"] = nc
    return nc


def make_in_maps(in_features, Wq, Wk, Wv, Wo):
    x = np.asarray(in_features, dtype=np.float32).astype(np.float16)
    # K projection fused into Q: scores = x@(Wq.T@Wk)@x.T
    A = np.ascontiguousarray(
        (np.asarray(Wq, np.float32).T @ np.asarray(Wk, np.float32)).astype(
            np.float16
        )
    )
    # V and output projections fused: out = P@(x@Wv.T)@Wo.T = (P@x)@M
    M = np.ascontiguousarray(
        (np.asarray(Wv, np.float32).T @ np.asarray(Wo, np.float32).T).astype(
            np.float16
        )
    )
    tri = np.tril(np.ones((128, 128), np.float16)).T  # keep iff q_local >= k_local
    onesq = np.ones((128, 128), np.float16)
    zer = np.zeros((128, 128), np.float16)
    in_maps = []
    for c in range(8):
        b, par = divmod(c, 2)
        xkvT = np.ascontiguousarray(x[b].T)
        blocks = [4 * t + 2 * h + par for t in range(4) for h in range(2)]
        xqT = np.ascontiguousarray(
            np.concatenate([x[b, 128 * j : 128 * (j + 1), :] for j in blocks]).T
        )
        if par == 0:
            m0, m1 = tri, zer
        else:
            m0, m1 = onesq, tri
        in_maps.append(
            {
                "xkvT": xkvT,
                "xqT": xqT,
                "xkv": np.ascontiguousarray(x[b]),
                "wqT": A,
                "woT": M,
                "m0f": np.ascontiguousarray(np.concatenate([m0, onesq], axis=1)),
                "m1f": np.ascontiguousarray(np.concatenate([m1, onesq], axis=1)),
            }
        )
    return in_maps


def kernel(in_features, Wq, Wk, Wv, Wo, trace=False):
    from concourse.bass_utils import run_bass_kernel_spmd

    nc = _build()
    in_maps = make_in_maps(in_features, Wq, Wk, Wv, Wo)
    res = run_bass_kernel_spmd(nc, in_maps, core_ids=list(range(8)), trace=trace)
    out = np.empty((B, S, DV), np.float32)
    for c in range(8):
        b, par = divmod(c, 2)
        o = res.results[c]["out"]
        for t in range(4):
            for h in range(2):
                j = 4 * t + 2 * h + par
                r = (2 * t + h) * 128
                out[b, 128 * j : 128 * (j + 1), :] = o[r : r + 128, :]
    if trace:
        kernel.last_result = res
    return out


# revision 33
# speedup vs baseline: 1.8597x; 1.1983x over previous
"""Causal single-head attention (d_model-wide) for Trainium2, 8-core SPMD.

Problem: B=4, S=2048, D=1024; Q/K/V = x@W{q,k,v}.T; scores=Q@K.T (causal,
scale 1/sqrt(D)); out = softmax(scores)@V @ Wo.T -> [B, S, 64].

Sharding (profile-uniform causal blocks): each batch b has 16 query blocks of
128 rows; block j needs j+1 key chunks (causal). Core 2b gets the odd-count
blocks {0,2,...,14}, core 2b+1 the even-count blocks {1,3,...,15}; counts
round up to the even profile P in {2,4,...,16}, so EVERY core runs the same
chunk schedule (one SPMD program) while skipping all score blocks above the
causal diagonal. Blocks pair as (P, P+2) into 256-wide matmul tiles: chunks
< P serve both halves (N=256), the last two chunks serve only the second
half (N=128). The only per-core differences are data: which x rows feed Q,
and two [128,256] fp16 multiplicative masks m0/m1 (triangle/ones/zeros)
applied post-exp at each block's last two chunks.

On-device layout (no transposes anywhere; host pre-transposes x and W):
  KT[e, k] = sum_d WkT[d,e] xT[d,k]      (lhsT=WkT tile, rhs=xT tile)
  QT[e, q] likewise; V[k, e] = sum_d xT[d,k] WvT[d,e] (lhsT=xT, rhs=WvT)
  ST[k, q] = sum_e KT[e,k] QT[e,q]       (scores, transposed; PSUM f32)
  PT[k, q] = exp(ST/32) * mask           (ACT exp, fp16; no max-subtraction:
                                          |ST|/32 <~ 8 so exp is fp32-safe)
  ctxT[e, q] += sum_k V[k,e] PT[k,q]     (PSUM f32 over k chunks; 2 e-slices
                                          per 2KB bank: one start/stop per
                                          bank since start zeroes the bank)
  sums[q]   += sum_k PT[k,q]             (ones-matmul into PSUM)
  out[q, v] = (sum_e ctxT[e,q] WoT[e,v]) * (1/sums[q])

Everything is fp16 storage (inputs converted on host) with fp32 PSUM
accumulation; measured rel err vs the fp32 reference is ~5e-4.
"""

import numpy as np

B, S, D, DV = 4, 2048, 1024, 64
NQ = S // 2          # queries per core
ET = D // 128        # 8 e-tiles
DT = D // 128        # 8 d-tiles
KT_N = S // 128      # 16 key tiles of 128
PCHUNK = 512         # projection free-dim chunk
QB = 256             # query block (free dim of attention matmuls)
N_QB = NQ // QB      # 4
SCALE = 1.0 / 32.0   # 1/sqrt(D)
NEG = -1.0e30

_cache = {}


def _patch_drain_split(tile_mod, mybir, ScopedClock):
    """This walrus build accepts only ONE sync-wait on the kernel-tail SP
    Drain; split the waits across multiple drain instructions."""
    if getattr(tile_mod.TileContext, "_drain_split_patched", False):
        return

    def _patched(self, tick_clock, wait_clock):
        nc = self.nc
        drain_inst = nc.sync.drain()
        wait_clock.add_sem_waits(
            drain_inst.ins, ScopedClock({None: tick_clock.global_clock})
        )
        ins = drain_inst.ins
        waits = list(ins.sync_info.on_wait or [])
        if len(waits) > 1:
            ins.sync_info = mybir.SyncInfo(
                on_wait=waits[:1], on_update=list(ins.sync_info.on_update or [])
            )
            for w in waits[1:]:
                extra = nc.sync.drain()
                extra.ins.sync_info = mybir.SyncInfo(on_wait=[w], on_update=[])
        nc.all_engine_barrier()
        assert self.sems is not None
        popped = nc._tile_sem_poison_stack.pop()
        assert popped is self._sem_poison
        nc.clear_and_free_semaphores(list(self.sems.allocated().values()))
        nc.all_engine_barrier()

    tile_mod.TileContext._drain_and_barrier = _patched
    tile_mod.TileContext._drain_split_patched = True


def _split_multiwait_bir(bir_json):
    """This walrus build accepts only one sync-wait per instruction. Rewrite
    the BIR so any instruction with N>1 waits is preceded by N-1 single-wait
    Drain instructions on the same engine (engine streams execute in block
    order, so the waits are enforced before the instruction issues)."""
    import json

    m = json.loads(bir_json)
    changed = False
    for fn in m.get("functions", []):
        for blk in fn.get("blocks", []):
            insts = blk.get("instructions", [])
            out = []
            for inst in insts:
                si = inst.get("sync_info")
                waits = (si or {}).get("on_wait") or []
                if len(waits) > 1:
                    changed = True
                    for i, w in enumerate(waits[:-1]):
                        out.append(
                            {
                                "name": f"{inst['name']}-sw{i}",
                                "opcode": "Drain",
                                "engine": inst["engine"],
                                "ins": [],
                                "outs": [],
                                "is_reset_sema": False,
                                "sync_info": {"on_wait": [w], "on_update": []},
                                "debug": inst.get("debug"),
                            }
                        )
                    si["on_wait"] = [waits[-1]]
                out.append(inst)
            blk["instructions"] = out
    if not changed:
        return bir_json
    return json.dumps(m).encode()


def _patch_compile_hook():
    """Route every BIR compile through _split_multiwait_bir."""
    import concourse.bass_utils as bu

    if getattr(bu, "_multiwait_patched", False):
        return
    orig = bu.compile_bir_kernel

    def wrapped(bir_json, tmpdir, neff_name="file.neff"):
        return orig(_split_multiwait_bir(bir_json), tmpdir, neff_name)

    bu.compile_bir_kernel = wrapped
    bu._multiwait_patched = True
    try:
        import concourse.bass2jax as b2j

        b2j.compile_bir_kernel = wrapped
    except Exception:
        pass


def _install_ntff_hook():
    """Provide antenv.axon_hooks (absent in this image) so that
    run_bass_kernel_spmd(trace=True) can profile through axon."""
    import sys
    import types

    if "antenv.axon_hooks" in sys.modules:
        return
    try:
        import trn_agent_boot.trn_boot as tb

        mod = types.ModuleType("antenv.axon_hooks")
        _hook = [None]
        mod.set_axon_ntff_profile_hook = lambda h: _hook.__setitem__(0, h)
        mod.get_axon_ntff_profile_hook = lambda: _hook[0]
        sys.modules["antenv.axon_hooks"] = mod
        mod.set_axon_ntff_profile_hook(
            tb._ntff_profile_via_ctypes("/opt/axon/libaxon_pjrt.so")
        )
    except Exception:
        pass


def _build():
    if "nc" in _cache:
        return _cache["nc"]

    import concourse.bass as bass
    import concourse.mybir as mybir
    import concourse.tile as tile
    from concourse.vector_clock import ScopedClock

    _patch_drain_split(tile, mybir, ScopedClock)
    _patch_compile_hook()
    _install_ntff_hook()

    f32 = mybir.dt.float32
    f32r = mybir.dt.float32r
    f16 = mybir.dt.float16

    nc = bass.Bass()
    xkvT = nc.dram_tensor("xkvT", [D, S], f16, kind="ExternalInput")
    xqT = nc.dram_tensor("xqT", [D, NQ], f16, kind="ExternalInput")
    xkv = nc.dram_tensor("xkv", [S, D], f16, kind="ExternalInput")
    wqT = nc.dram_tensor("wqT", [D, D], f16, kind="ExternalInput")
    woT = nc.dram_tensor("woT", [D, DV], f16, kind="ExternalInput")
    m0f = nc.dram_tensor("m0f", [128, QB], f16, kind="ExternalInput")
    m1f = nc.dram_tensor("m1f", [128, QB], f16, kind="ExternalInput")
    out = nc.dram_tensor("out", [NQ, DV], f32, kind="ExternalOutput")

    with tile.TileContext(nc) as tc:
        with (
            tc.tile_pool(name="kt", bufs=1) as ktp,
            tc.tile_pool(name="v", bufs=1) as vp,
            tc.tile_pool(name="qt", bufs=1) as qtp,
            tc.tile_pool(name="small", bufs=1) as small,
            tc.tile_pool(name="attn_sb", bufs=4) as attn_sb,
            tc.tile_pool(name="ctxsb", bufs=2) as ctxsbp,
            tc.tile_pool(name="osb", bufs=2) as osbp,
        ):
            # persistent fp16 operand stores
            v_sb = [vp.tile([128, D], f16, tag=f"v{k}", name=f"v{k}") for k in range(KT_N)]
            qt_sb = [qtp.tile([128, NQ], f16, tag=f"qt{e}", name=f"qt{e}") for e in range(ET)]

            xk_sb = [ktp.tile([128, S], f16, tag=f"xk{d}", name=f"xk{d}") for d in range(DT)]
            ones_sb = small.tile([128, 1], f16, name="ones")
            nc.vector.memset(ones_sb, 1.0)
            m0_sb = small.tile([128, QB], f16, name="m0")
            nc.gpsimd.dma_start(out=m0_sb, in_=m0f[:, :])
            m1_sb = small.tile([128, QB], f16, name="m1")
            nc.gpsimd.dma_start(out=m1_sb, in_=m1f[:, :])
            wo_sb = small.tile([128, ET, DV], f16, name="wo")
            nc.gpsimd.dma_start(
                out=wo_sb, in_=woT.rearrange("(t p) v -> p t v", p=128)
            )

            # ---------- projections ----------
            # x and all weights fully SBUF-resident in fp16: every tile is
            # written exactly once by DMA (no WAR hazards on input buffers).
            with (
                tc.tile_pool(name="w", bufs=1) as wp,
                tc.tile_pool(name="xin", bufs=1) as xin,
                tc.tile_pool(name="pproj", bufs=4, space="PSUM") as pproj,
            ):
                xq_sb = [xin.tile([128, NQ], f16, tag=f"xq{d}", name=f"xq{d}") for d in range(DT)]
                wq_t = [wp.tile([128, D], f16, tag=f"wq{d}", name=f"wq{d}") for d in range(DT)]
                # DMA order tuned for earliest PE start. Descriptor prep is
                # ~600ns each and serial per trigger engine, so the critical
                # first tiles are split across BOTH HWDGE trigger engines
                # (SP carries wk, ACT carries xk chunk 0) and interleaved.
                # QT proj is now the first PE work: its inputs (A on SP,
                # xq on ACT) load first; then xk/x-natural interleave in the
                # order attention consumes them.
                for d in range(DT):
                    ds = slice(d * 128, (d + 1) * 128)
                    nc.sync.dma_start(out=wq_t[d], in_=wqT[ds, :])
                    nc.scalar.dma_start(out=xq_sb[d], in_=xqT[ds, :])
                for c in range(S // PCHUNK):
                    cs = slice(c * PCHUNK, (c + 1) * PCHUNK)
                    for d in range(DT):
                        ds = slice(d * 128, (d + 1) * 128)
                        eng = nc.sync if d % 2 == 0 else nc.scalar
                        eng.dma_start(out=xk_sb[d][:, cs], in_=xkvT[ds, cs])
                    for kg in range(4 * c, 4 * (c + 1)):
                        eng = nc.sync if kg % 2 == 0 else nc.scalar
                        eng.dma_start(
                            out=v_sb[kg], in_=xkv[kg * 128 : (kg + 1) * 128, :]
                        )

                def qt_chunk(c):
                    cs = slice(c * PCHUNK, (c + 1) * PCHUNK)
                    for e in range(ET):
                        ps = pproj.tile([128, PCHUNK], f32, tag="pproj", name="pproj")
                        for d in range(DT):
                            nc.tensor.matmul(
                                ps,
                                lhsT=wq_t[d][:, e * 128 : (e + 1) * 128],
                                rhs=xq_sb[d][:, cs],
                                start=(d == 0),
                                stop=(d == DT - 1),
                            )
                        nc.vector.tensor_copy(qt_sb[e][:, cs], ps)

                qt_chunk(0)
                qt_chunk(1)

            # ---------- attention ----------
            with (
                tc.tile_pool(name="pst", bufs=2, space="PSUM") as pst,
                tc.tile_pool(name="pctx", bufs=1, space="PSUM") as pctx,
                tc.tile_pool(name="psum_s", bufs=1, space="PSUM") as psums,
                tc.tile_pool(name="pout", bufs=1, space="PSUM") as pout,
            ):
                for t in range(N_QB):  # pairs: profiles (4t+2, 4t+4)
                    Pa = 4 * t + 2
                    qs = slice(t * QB, (t + 1) * QB)
                    qs1 = slice(t * QB + 128, (t + 1) * QB)
                    ctx = pctx.tile([128, ET, QB], mybir.dt.float32, tag="ctx", name="ctx")
                    sums = psums.tile([128, 2], mybir.dt.float32, tag="sums", name="sums")
                    # joint chunks: both halves of the pair attend (N=256)
                    for kc in range(Pa):
                        st = pst.tile([128, QB], mybir.dt.float32, tag="st", name="st")
                        for e in range(ET):
                            nc.tensor.matmul(
                                st,
                                lhsT=xk_sb[e][:, kc * 128 : (kc + 1) * 128],
                                rhs=qt_sb[e][:, qs],
                                start=(e == 0),
                                stop=(e == ET - 1),
                            )
                        pt = attn_sb.tile([128, QB], f16, tag="pt", name="pt")
                        nc.scalar.activation(
                            pt, st, mybir.ActivationFunctionType.Exp, scale=SCALE
                        )
                        if kc == Pa - 2:
                            nc.vector.tensor_mul(pt, pt, m0_sb)
                        elif kc == Pa - 1:
                            nc.vector.tensor_mul(pt, pt, m1_sb)
                        for h in range(2):
                            nc.tensor.matmul(
                                sums[:, h : h + 1],
                                lhsT=pt[:, h * 128 : (h + 1) * 128],
                                rhs=ones_sb,
                                start=(kc == 0 and h == 0),
                                stop=False,
                            )
                        # ctx: 2 e-slices share a PSUM bank; start zeroes the
                        # whole bank so only the first slice starts it.
                        for e in range(ET):
                            nc.tensor.matmul(
                                ctx[:, e, :],
                                lhsT=v_sb[kc][:, e * 128 : (e + 1) * 128],
                                rhs=pt,
                                start=(kc == 0 and e % 2 == 0),
                                stop=False,
                            )
                    # tail chunks: only the pair's second half attends (N=128)
                    for kc in (Pa, Pa + 1):
                        st = pst.tile([128, QB], mybir.dt.float32, tag="st", name="st")
                        for e in range(ET):
                            nc.tensor.matmul(
                                st[:, :128],
                                lhsT=xk_sb[e][:, kc * 128 : (kc + 1) * 128],
                                rhs=qt_sb[e][:, qs1],
                                start=(e == 0),
                                stop=(e == ET - 1),
                            )
                        pt = attn_sb.tile([128, QB], f16, tag="pt", name="pt")
                        nc.scalar.activation(
                            pt[:, :128],
                            st[:, :128],
                            mybir.ActivationFunctionType.Exp,
                            scale=SCALE,
                        )
                        msk = m0_sb if kc == Pa else m1_sb
                        nc.vector.tensor_mul(pt[:, :128], pt[:, :128], msk[:, :128])
                        nc.tensor.matmul(
                            sums[:, 1:2],
                            lhsT=pt[:, :128],
                            rhs=ones_sb,
                            start=False,
                            stop=(kc == Pa + 1),
                        )
                        for e in range(ET):
                            nc.tensor.matmul(
                                ctx[:, e, 128:QB],
                                lhsT=v_sb[kc][:, e * 128 : (e + 1) * 128],
                                rhs=pt[:, :128],
                                start=False,
                                stop=(kc == Pa + 1 and e % 2 == 1),
                            )
                    # epilogue for this pair
                    ctxsb = ctxsbp.tile([128, ET, QB], f16, tag="ctxsb", name="ctxsb")
                    rec = attn_sb.tile([128, 2], mybir.dt.float32, tag="rec", name="rec")
                    nc.vector.reciprocal(rec, sums)
                    for h in range(2):
                        for e in range(ET):
                            nc.vector.tensor_copy(
                                ctxsb[:, e, h * 128 : (h + 1) * 128],
                                ctx[:, e, h * 128 : (h + 1) * 128],
                            )
                        ops = pout.tile([128, DV], mybir.dt.float32, tag="ops", name="ops")
                        for e in range(ET):
                            nc.tensor.matmul(
                                ops,
                                lhsT=ctxsb[:, e, h * 128 : (h + 1) * 128],
                                rhs=wo_sb[:, e, :],
                                start=(e == 0),
                                stop=(e == ET - 1),
                            )
                        o = osbp.tile([128, DV], mybir.dt.float32, tag="o", name="o")
                        nc.scalar.activation(
                            o,
                            ops,
                            mybir.ActivationFunctionType.Copy,
                            bias=0.0,
                            scale=rec[:, h : h + 1],
                        )
                        r0 = t * QB + h * 128
                        nc.sync.dma_start(out=out[r0 : r0 + 128, :], in_=o)

    _cache["nc"] = nc
    return nc


def make_in_maps(in_features, Wq, Wk, Wv, Wo):
    x = np.asarray(in_features, dtype=np.float32).astype(np.float16)
    # K projection fused into Q: scores = x@(Wq.T@Wk)@x.T
    A = np.ascontiguousarray(
        (np.asarray(Wq, np.float32).T @ np.asarray(Wk, np.float32)).astype(
            np.float16
        )
    )
    # V and output projections fused: out = P@(x@Wv.T)@Wo.T = (P@x)@M
    M = np.ascontiguousarray(
        (np.asarray(Wv, np.float32).T @ np.asarray(Wo, np.float32).T).astype(
            np.float16
        )
    )
    tri = np.tril(np.ones((128, 128), np.float16)).T  # keep iff q_local >= k_local
    onesq = np.ones((128, 128), np.float16)
    zer = np.zeros((128, 128), np.float16)
    in_maps = []
    for c in range(8):
        b, par = divmod(c, 2)
        xkvT = np.ascontiguousarray(x[b].T)
        blocks = [4 * t + 2 * h + par for t in range(4) for h in range(2)]
        xqT = np.ascontiguousarray(
            np.concatenate([x[b, 128 * j : 128 * (j + 1), :] for j in blocks]).T
        )
        if par == 0:
            m0, m1 = tri, zer
        else:
            m0, m1 = onesq, tri
        in_maps.append(
            {
                "xkvT": xkvT,
                "xqT": xqT,
                "xkv": np.ascontiguousarray(x[b]),
                "wqT": A,
                "woT": M,
                "m0f": np.ascontiguousarray(np.concatenate([m0, onesq], axis=1)),
                "m1f": np.ascontiguousarray(np.concatenate([m1, onesq], axis=1)),
            }
        )
    return in_maps


def kernel(in_features, Wq, Wk, Wv, Wo, trace=False):
    from concourse.bass_utils import run_bass_kernel_spmd

    nc = _build()
    in_maps = make_in_maps(in_features, Wq, Wk, Wv, Wo)
    res = run_bass_kernel_spmd(nc, in_maps, core_ids=list(range(8)), trace=trace)
    out = np.empty((B, S, DV), np.float32)
    for c in range(8):
        b, par = divmod(c, 2)
        o = res.results[c]["out"]
        for t in range(4):
            for h in range(2):
                j = 4 * t + 2 * h + par
                r = (2 * t + h) * 128
                out[b, 128 * j : 128 * (j + 1), :] = o[r : r + 128, :]
    if trace:
        kernel.last_result = res
    return out
